# revision 6
# baseline (speedup 1.0000x reference)
"""Trainium2 Bass kernel for BertSelfAttention (B=4, L=2048, D=1024, H=16).

Sharding: 8 cores = 4 batches x 2 head-groups (8 heads each). Each core
computes QKV projection (+RoPE) for its heads, attention transposed
(S^T = K^T.T @ Q^T per head, softmax sums via a ones-column appended to V),
and a partial output projection over its 512 attn dims. Host sums the two
partials per batch.

All activations flow in "transposed" [feature, token] layout so no on-device
transposes are needed; weight/activation transposes are done host-side as
part of sharding.
"""

import sys

sys.path.insert(0, "/opt/trn_rl_repo")

from contextlib import ExitStack

import numpy as np

B, L, D, H, DH = 4, 2048, 1024, 16, 64
HL = 8          # local heads per core
EQK = 512       # q/k/v feature dims per core (HL * DH)
NCORES = 8
P = 128
TT = L // P     # 16 token tiles
DC = D // P     # 8 contraction chunks
KT = L // P     # 16 key tiles
QH = 2          # q halves
QHW = L // QH   # 1024

_CACHE = {}


def _build_bass():
    import concourse.tile as tile
    from concourse import bacc, mybir

    f32 = mybir.dt.float32
    f16 = mybir.dt.float16
    f32r = mybir.dt.float32r
    AF = mybir.ActivationFunctionType
    ALU = mybir.AluOpType

    nc = bacc.Bacc("TRN2", target_bir_lowering=False, debug=False)

    hid_d = nc.dram_tensor("hid", [D, L], f32, kind="ExternalInput").ap()
    wq_d = nc.dram_tensor("wq", [D, EQK], f32, kind="ExternalInput").ap()
    wk_d = nc.dram_tensor("wk", [D, EQK], f32, kind="ExternalInput").ap()
    wv_d = nc.dram_tensor("wv", [D, EQK], f32, kind="ExternalInput").ap()
    wo_d = nc.dram_tensor("wo", [EQK, D], f32, kind="ExternalInput").ap()
    cos_d = nc.dram_tensor("cosb", [P, L], f32, kind="ExternalInput").ap()
    sin_d = nc.dram_tensor("sinb", [P, L], f32, kind="ExternalInput").ap()
    out_d = nc.dram_tensor("out", [L, D], f32, kind="ExternalOutput").ap()

    with tile.TileContext(nc) as tc, ExitStack() as ctx:
        # ---- persistent pools (live through the whole kernel) ----
        persist = ctx.enter_context(tc.tile_pool(name="persist", bufs=1))
        qh_sb = [persist.tile([P, L], f16, tag=f"qh{i}", name=f"qh{i}") for i in range(4)]
        kh_sb = [persist.tile([P, L], f16, tag=f"kh{i}", name=f"kh{i}") for i in range(4)]
        VSLOT = DH + 1  # 65: V columns + trailing ones column per head
        v_sb = persist.tile([P, TT, HL * VSLOT], f16, tag="v")

        # ---- projection-phase pools (closed before attention) ----
        with tc.tile_pool(name="projsb", bufs=1) as projsb, \
             tc.tile_pool(name="grouped", bufs=4) as grouped, \
             tc.tile_pool(name="ropetmp", bufs=4) as ropetmp, \
             tc.tile_pool(name="projps", bufs=4, space="PSUM") as projps:

            hid_sb = projsb.tile([P, DC, L], f32r, tag="hid")
            nc.sync.dma_start(hid_sb[:], hid_d.rearrange("(c p) t -> p c t", p=P).bitcast(f32r))
            wq_sb = projsb.tile([P, DC, EQK], f32r, tag="wq")
            nc.sync.dma_start(wq_sb[:], wq_d.rearrange("(c p) e -> p c e", p=P).bitcast(f32r))
            wk_sb = projsb.tile([P, DC, EQK], f32r, tag="wk")
            nc.sync.dma_start(wk_sb[:], wk_d.rearrange("(c p) e -> p c e", p=P).bitcast(f32r))
            wv_sb = projsb.tile([P, DC, EQK], f32r, tag="wv")
            nc.sync.dma_start(wv_sb[:], wv_d.rearrange("(c p) e -> p c e", p=P).bitcast(f32r))
            cos_sb = projsb.tile([P, L], f32, tag="cos")
            nc.sync.dma_start(cos_sb[:], cos_d[:])
            sin_sb = projsb.tile([P, L], f32, tag="sin")
            nc.sync.dma_start(sin_sb[:], sin_d[:])

            # ones columns of V' (set once; V copies fill the rest)
            ones_ap = v_sb[:].rearrange("p t (h w) -> p t h w", w=VSLOT)[:, :, :, DH:DH + 1]
            nc.vector.memset(ones_ap, 1.0)

            def qk_proj(w_sb, dst_tiles):
                # e-tiles: 0 = x1 h0-3, 1 = x1 h4-7, 2 = x2 h0-3, 3 = x2 h4-7
                for half in range(2):
                    g1, g2 = half, 2 + half
                    for tci in range(4):
                        tsl = slice(tci * 512, (tci + 1) * 512)
                        ps1 = projps.tile([P, 512], f32, tag="pps")
                        ps2 = projps.tile([P, 512], f32, tag="pps")
                        for dc in range(DC):
                            nc.tensor.matmul(
                                ps1[:], w_sb[:, dc, g1 * P:(g1 + 1) * P],
                                hid_sb[:, dc, tsl],
                                start=(dc == 0), stop=(dc == DC - 1))
                        for dc in range(DC):
                            nc.tensor.matmul(
                                ps2[:], w_sb[:, dc, g2 * P:(g2 + 1) * P],
                                hid_sb[:, dc, tsl],
                                start=(dc == 0), stop=(dc == DC - 1))
                        cs, sn = cos_sb[:, tsl], sin_sb[:, tsl]
                        gx1 = grouped.tile([P, 512], f16, tag="gx")
                        gx2 = grouped.tile([P, 512], f16, tag="gx")
                        t1 = ropetmp.tile([P, 512], f32, tag="rt")
                        t2 = ropetmp.tile([P, 512], f32, tag="rt")
                        nc.vector.tensor_mul(t1[:], ps1[:], cs)
                        nc.vector.tensor_mul(t2[:], ps2[:], sn)
                        nc.vector.tensor_add(gx1[:], t1[:], t2[:])
                        t3 = ropetmp.tile([P, 512], f32, tag="rt")
                        t4 = ropetmp.tile([P, 512], f32, tag="rt")
                        nc.vector.tensor_mul(t3[:], ps2[:], cs)
                        nc.vector.tensor_mul(t4[:], ps1[:], sn)
                        nc.vector.tensor_sub(gx2[:], t3[:], t4[:])
                        # repack: per-head contiguous rows [y1(32) | y2(32)]
                        for j in range(4):
                            h = half * 4 + j
                            dst = dst_tiles[h // 2]
                            rb = (h % 2) * DH
                            nc.sync.dma_start(dst[rb:rb + 32, tsl], gx1[j * 32:(j + 1) * 32, :])
                            nc.sync.dma_start(dst[rb + 32:rb + 64, tsl], gx2[j * 32:(j + 1) * 32, :])

            qk_proj(wq_sb, qh_sb)
            qk_proj(wk_sb, kh_sb)

            # V projection: [t, e] layout, fp16, into per-head 65-wide slots
            for tt in range(TT):
                psv = projps.tile([P, 512], f32, tag="pps")
                for dc in range(DC):
                    nc.tensor.matmul(
                        psv[:], hid_sb[:, dc, tt * P:(tt + 1) * P],
                        wv_sb[:, dc, :],
                        start=(dc == 0), stop=(dc == DC - 1))
                dst = v_sb[:, tt].rearrange("p (h w) -> p h w", w=VSLOT)[:, :, 0:DH]
                nc.vector.tensor_copy(dst, psv[:].rearrange("p (h w) -> p h w", w=DH))

        # ---- attention + output pools ----
        with tc.tile_pool(name="attnsb", bufs=1) as attnsb, \
             tc.tile_pool(name="ppool", bufs=3) as ppool, \
             tc.tile_pool(name="divtmp", bufs=2) as divtmp, \
             tc.tile_pool(name="osb", bufs=4) as opool:

            attnc = [attnsb.tile([P, L], f32r, tag=f"attnc{i}", name=f"attnc{i}") for i in range(4)]
            wo_sb = attnsb.tile([P, 4, D], f32r, tag="wo")
            nc.sync.dma_start(wo_sb[:], wo_d.rearrange("(c p) e -> p c e", p=P).bitcast(f32r))

            attn_ps = ExitStack()
            sps = attn_ps.enter_context(tc.tile_pool(name="sps", bufs=2, space="PSUM"))
            pvps = attn_ps.enter_context(tc.tile_pool(name="pvps", bufs=2, space="PSUM"))

            for qh in range(QH):
                for h in range(HL):
                    rb = (h % 2) * DH
                    q_ap = qh_sb[h // 2][rb:rb + DH, qh * QHW:(qh + 1) * QHW]
                    k_tile = kh_sb[h // 2]
                    pv = pvps.tile([DH + 1, QHW], f32, tag="pv")
                    p_tiles = [None] * KT
                    for ki in range(KT + 1):
                        if ki < KT:
                            s = sps.tile([P, QHW], f32, tag="s")
                            for qc in range(2):
                                nc.tensor.matmul(
                                    s[:, qc * 512:(qc + 1) * 512],
                                    k_tile[rb:rb + DH, ki * P:(ki + 1) * P],
                                    q_ap[:, qc * 512:(qc + 1) * 512],
                                    start=True, stop=True)
                        if ki >= 1:
                            pm = p_tiles[ki - 1]
                            vsl = v_sb[:, ki - 1, h * VSLOT:(h + 1) * VSLOT]
                            for qc in range(2):
                                nc.tensor.matmul(
                                    pv[:, qc * 512:(qc + 1) * 512],
                                    vsl, pm[:, qc * 512:(qc + 1) * 512],
                                    start=(ki - 1 == 0), stop=(ki - 1 == KT - 1))
                        if ki < KT:
                            p = ppool.tile([P, QHW], f16, tag="p")
                            nc.scalar.activation(p[:], s[:], AF.Exp)
                            p_tiles[ki] = p
                    # softmax denominators: row DH of pv holds sum_k exp
                    rec = divtmp.tile([DH + 1, QHW], f32, tag="rec")
                    nc.vector.reciprocal(rec[DH:DH + 1, :], pv[DH:DH + 1, :])
                    r0 = divtmp.tile([1, QHW], f32, tag="r0")
                    nc.sync.dma_start(r0[:], rec[DH:DH + 1, :])
                    recb = divtmp.tile([DH, QHW], f32, tag="recb")
                    nc.gpsimd.partition_broadcast(recb[:], r0[:], channels=DH)
                    at = divtmp.tile([DH, QHW], f32r, tag="at")
                    nc.vector.tensor_tensor(at[:], pv[0:DH, :], recb[:], ALU.mult)
                    nc.sync.dma_start(
                        attnc[h // 2][rb:rb + DH, qh * QHW:(qh + 1) * QHW], at[:])

            # output projection: out[t, e] partial = attnc^T @ wo
            attn_ps.close()
            with tc.tile_pool(name="wops", bufs=4, space="PSUM") as wops:
                for tt in range(TT):
                    for ec in range(2):
                        po = wops.tile([P, 512], f32, tag="po")
                        for dci in range(4):
                            nc.tensor.matmul(
                                po[:], attnc[dci][:, tt * P:(tt + 1) * P],
                                wo_sb[:, dci, ec * 512:(ec + 1) * 512],
                                start=(dci == 0), stop=(dci == 3))
                        ob = opool.tile([P, 512], f32, tag="ob")
                        nc.scalar.copy(ob[:], po[:])
                        nc.sync.dma_start(
                            out_d[tt * P:(tt + 1) * P, ec * 512:(ec + 1) * 512], ob[:])

    nc.compile()
    return nc


def _host_prep(hidden_states, sin, cos, Wqkv, Wo):
    hidden = np.asarray(hidden_states, dtype=np.float32)
    sin = np.asarray(sin, dtype=np.float32)
    cos = np.asarray(cos, dtype=np.float32)
    Wqkv = np.asarray(Wqkv, dtype=np.float32)
    Wo = np.asarray(Wo, dtype=np.float32)

    Wq, Wk, Wv = Wqkv[0:D], Wqkv[D:2 * D], Wqkv[2 * D:3 * D]
    cos32 = np.ascontiguousarray(cos[0, :, 0, :].T)  # [32, L]
    sin32 = np.ascontiguousarray(sin[0, :, 0, :].T)
    cosb = np.ascontiguousarray(np.tile(cos32, (4, 1)))  # [128, L]
    sinb = np.ascontiguousarray(np.tile(sin32, (4, 1)))

    hid_t = [np.ascontiguousarray(hidden[b].T) for b in range(B)]

    in_maps = []
    for core in range(NCORES):
        b, hg = core // 2, core % 2
        heads = range(hg * HL, (hg + 1) * HL)

        def grouped_t(W, scale=1.0):
            rows = []
            for xh in (0, 1):
                for h in heads:
                    rows.append(W[h * DH + xh * 32: h * DH + xh * 32 + 32])
            g = np.concatenate(rows, 0)  # [512, D]
            return np.ascontiguousarray(g.T * scale)  # [D, 512]

        wq_t = grouped_t(Wq, scale=1.0 / np.sqrt(DH))
        wk_t = grouped_t(Wk)
        wv_g = np.concatenate([Wv[h * DH:(h + 1) * DH] for h in heads], 0)
        wv_t = np.ascontiguousarray(wv_g.T)
        wo_t = np.ascontiguousarray(Wo.T[hg * EQK:(hg + 1) * EQK, :])

        in_maps.append({
            "hid": hid_t[b], "wq": wq_t, "wk": wk_t, "wv": wv_t,
            "wo": wo_t, "cosb": cosb, "sinb": sinb,
        })
    return in_maps


def kernel(hidden_states, mask, sin, cos, Wqkv, Wo, _trace=False, _tmpdir=None):
    from concourse.bass_utils import run_bass_kernel_spmd

    if "nc" not in _CACHE:
        _CACHE["nc"] = _build_bass()
    nc = _CACHE["nc"]

    in_maps = _host_prep(hidden_states, sin, cos, Wqkv, Wo)
    kwargs = {}
    if _trace:
        kwargs = dict(trace=True, trace_cores=list(range(NCORES)), tmpdir=_tmpdir)
    res = run_bass_kernel_spmd(nc, in_maps, core_ids=list(range(NCORES)), **kwargs)
    _CACHE["last_result"] = res

    out = np.empty((B, L, D), dtype=np.float32)
    for b in range(B):
        out[b] = res.results[2 * b]["out"] + res.results[2 * b + 1]["out"]
    return out


# revision 8
# speedup vs baseline: 1.2177x; 1.2177x over previous
"""Trainium2 Bass kernel for BertSelfAttention (B=4, L=2048, D=1024, H=16).

Sharding: 8 cores = 4 batches x 2 head-groups (8 heads each). Each core
computes QKV projection (+RoPE) for its heads, attention transposed
(S^T = K^T.T @ Q^T per head, softmax sums via a ones-column appended to V),
and a partial output projection over its 512 attn dims. Host sums the two
partials per batch.

All activations flow in "transposed" [feature, token] layout so no on-device
transposes are needed; weight/activation transposes are done host-side as
part of sharding.
"""

import sys

sys.path.insert(0, "/opt/trn_rl_repo")

from contextlib import ExitStack

import numpy as np

B, L, D, H, DH = 4, 2048, 1024, 16, 64
HL = 8          # local heads per core
EQK = 512       # q/k/v feature dims per core (HL * DH)
NCORES = 8
P = 128
TT = L // P     # 16 token tiles
DC = D // P     # 8 contraction chunks
KT = L // P     # 16 key tiles
QH = 2          # q halves
QHW = L // QH   # 1024

_CACHE = {}


def _build_bass():
    import concourse.tile as tile
    from concourse import bacc, mybir

    f32 = mybir.dt.float32
    f16 = mybir.dt.float16
    f32r = mybir.dt.float32r
    AF = mybir.ActivationFunctionType
    ALU = mybir.AluOpType

    nc = bacc.Bacc("TRN2", target_bir_lowering=False, debug=False)

    hid_d = nc.dram_tensor("hid", [D, L], f16, kind="ExternalInput").ap()
    wq_d = nc.dram_tensor("wq", [D, EQK], f16, kind="ExternalInput").ap()
    wk_d = nc.dram_tensor("wk", [D, EQK], f16, kind="ExternalInput").ap()
    wv_d = nc.dram_tensor("wv", [D, EQK], f16, kind="ExternalInput").ap()
    wo_d = nc.dram_tensor("wo", [EQK, D], f32, kind="ExternalInput").ap()
    cos_d = nc.dram_tensor("cosb", [P, L], f32, kind="ExternalInput").ap()
    sin_d = nc.dram_tensor("sinb", [P, L], f32, kind="ExternalInput").ap()
    out_d = nc.dram_tensor("out", [L, D], f32, kind="ExternalOutput").ap()

    with tile.TileContext(nc) as tc, ExitStack() as ctx:
        # ---- persistent pools (live through the whole kernel) ----
        persist = ctx.enter_context(tc.tile_pool(name="persist", bufs=1))
        qh_sb = [persist.tile([P, L], f16, tag=f"qh{i}", name=f"qh{i}") for i in range(4)]
        kh_sb = [persist.tile([P, L], f16, tag=f"kh{i}", name=f"kh{i}") for i in range(4)]
        VSLOT = DH + 1  # 65: V columns + trailing ones column per head
        v_sb = persist.tile([P, TT, HL * VSLOT], f16, tag="v")

        # ---- projection-phase pools (closed before attention) ----
        with tc.tile_pool(name="projsb", bufs=1) as projsb, \
             tc.tile_pool(name="grouped", bufs=4) as grouped, \
             tc.tile_pool(name="ropetmp", bufs=4) as ropetmp, \
             tc.tile_pool(name="projps", bufs=4, space="PSUM") as projps:

            hid_sb = projsb.tile([P, DC, L], f16, tag="hid")
            nc.sync.dma_start(hid_sb[:], hid_d.rearrange("(c p) t -> p c t", p=P))
            wq_sb = projsb.tile([P, DC, EQK], f16, tag="wq")
            nc.sync.dma_start(wq_sb[:], wq_d.rearrange("(c p) e -> p c e", p=P))
            wk_sb = projsb.tile([P, DC, EQK], f16, tag="wk")
            nc.sync.dma_start(wk_sb[:], wk_d.rearrange("(c p) e -> p c e", p=P))
            wv_sb = projsb.tile([P, DC, EQK], f16, tag="wv")
            nc.sync.dma_start(wv_sb[:], wv_d.rearrange("(c p) e -> p c e", p=P))
            cos_sb = projsb.tile([P, L], f32, tag="cos")
            nc.sync.dma_start(cos_sb[:], cos_d[:])
            sin_sb = projsb.tile([P, L], f32, tag="sin")
            nc.sync.dma_start(sin_sb[:], sin_d[:])

            # ones columns of V' (set once; V copies fill the rest)
            ones_ap = v_sb[:].rearrange("p t (h w) -> p t h w", w=VSLOT)[:, :, :, DH:DH + 1]
            nc.vector.memset(ones_ap, 1.0)

            def qk_proj(w_sb, dst_tiles):
                # e-tiles: 0 = x1 h0-3, 1 = x1 h4-7, 2 = x2 h0-3, 3 = x2 h4-7
                for half in range(2):
                    g1, g2 = half, 2 + half
                    for tci in range(4):
                        tsl = slice(tci * 512, (tci + 1) * 512)
                        ps1 = projps.tile([P, 512], f32, tag="pps")
                        ps2 = projps.tile([P, 512], f32, tag="pps")
                        for dc in range(DC):
                            nc.tensor.matmul(
                                ps1[:], w_sb[:, dc, g1 * P:(g1 + 1) * P],
                                hid_sb[:, dc, tsl],
                                start=(dc == 0), stop=(dc == DC - 1))
                        for dc in range(DC):
                            nc.tensor.matmul(
                                ps2[:], w_sb[:, dc, g2 * P:(g2 + 1) * P],
                                hid_sb[:, dc, tsl],
                                start=(dc == 0), stop=(dc == DC - 1))
                        cs, sn = cos_sb[:, tsl], sin_sb[:, tsl]
                        gx1 = grouped.tile([P, 512], f16, tag="gx")
                        gx2 = grouped.tile([P, 512], f16, tag="gx")
                        t1 = ropetmp.tile([P, 512], f32, tag="rt")
                        t2 = ropetmp.tile([P, 512], f32, tag="rt")
                        nc.vector.tensor_mul(t1[:], ps1[:], cs)
                        nc.vector.tensor_mul(t2[:], ps2[:], sn)
                        nc.vector.tensor_add(gx1[:], t1[:], t2[:])
                        t3 = ropetmp.tile([P, 512], f32, tag="rt")
                        t4 = ropetmp.tile([P, 512], f32, tag="rt")
                        nc.vector.tensor_mul(t3[:], ps2[:], cs)
                        nc.vector.tensor_mul(t4[:], ps1[:], sn)
                        nc.vector.tensor_sub(gx2[:], t3[:], t4[:])
                        # repack: per-head contiguous rows [y1(32) | y2(32)]
                        for j in range(4):
                            h = half * 4 + j
                            dst = dst_tiles[h // 2]
                            rb = (h % 2) * DH
                            nc.sync.dma_start(dst[rb:rb + 32, tsl], gx1[j * 32:(j + 1) * 32, :])
                            nc.sync.dma_start(dst[rb + 32:rb + 64, tsl], gx2[j * 32:(j + 1) * 32, :])

            qk_proj(wq_sb, qh_sb)
            qk_proj(wk_sb, kh_sb)

            # V projection: [t, e] layout, fp16, into per-head 65-wide slots
            for tt in range(TT):
                psv = projps.tile([P, 512], f32, tag="pps")
                for dc in range(DC):
                    nc.tensor.matmul(
                        psv[:], hid_sb[:, dc, tt * P:(tt + 1) * P],
                        wv_sb[:, dc, :],
                        start=(dc == 0), stop=(dc == DC - 1))
                dst = v_sb[:, tt].rearrange("p (h w) -> p h w", w=VSLOT)[:, :, 0:DH]
                nc.vector.tensor_copy(dst, psv[:].rearrange("p (h w) -> p h w", w=DH))

        # ---- attention + output pools ----
        with tc.tile_pool(name="attnsb", bufs=1) as attnsb, \
             tc.tile_pool(name="ppool", bufs=3) as ppool, \
             tc.tile_pool(name="divtmp", bufs=2) as divtmp, \
             tc.tile_pool(name="osb", bufs=4) as opool:

            attnc = [attnsb.tile([P, L], f32r, tag=f"attnc{i}", name=f"attnc{i}") for i in range(4)]
            wo_sb = attnsb.tile([P, 4, D], f32r, tag="wo")
            nc.sync.dma_start(wo_sb[:], wo_d.rearrange("(c p) e -> p c e", p=P).bitcast(f32r))

            attn_ps = ExitStack()
            sps = attn_ps.enter_context(tc.tile_pool(name="sps", bufs=2, space="PSUM"))
            pvps = attn_ps.enter_context(tc.tile_pool(name="pvps", bufs=2, space="PSUM"))

            first_unit = True
            for qh in range(QH):
                for h in range(HL):
                    rb = (h % 2) * DH
                    q_ap = qh_sb[h // 2][rb:rb + DH, qh * QHW:(qh + 1) * QHW]
                    k_tile = kh_sb[h // 2]
                    pv = pvps.tile([DH + 1, QHW], f32, tag="pv")
                    # HAM warm-up burst: dense dummy matmuls into the pv tile
                    # (overwritten by the real PV accumulation's start=True).
                    n_burst = 16 if first_unit else 6
                    first_unit = False
                    for _ in range(n_burst):
                        nc.tensor.matmul(
                            pv[:, 0:512], v_sb[:, 0, 0:DH + 1],
                            kh_sb[0][:, 0:512], start=True, stop=True)
                    p_tiles = [None] * KT
                    for ki in range(KT + 1):
                        if ki < KT:
                            s = sps.tile([P, QHW], f32, tag="s")
                            for qc in range(2):
                                nc.tensor.matmul(
                                    s[:, qc * 512:(qc + 1) * 512],
                                    k_tile[rb:rb + DH, ki * P:(ki + 1) * P],
                                    q_ap[:, qc * 512:(qc + 1) * 512],
                                    start=True, stop=True)
                        if ki >= 1:
                            pm = p_tiles[ki - 1]
                            vsl = v_sb[:, ki - 1, h * VSLOT:(h + 1) * VSLOT]
                            for qc in range(2):
                                nc.tensor.matmul(
                                    pv[:, qc * 512:(qc + 1) * 512], vsl,
                                    pm[:, qc * 512:(qc + 1) * 512],
                                    start=(ki - 1 == 0), stop=(ki - 1 == KT - 1))
                        if ki < KT:
                            p = ppool.tile([P, QHW], f16, tag="p")
                            nc.scalar.activation(p[:], s[:], AF.Exp)
                            p_tiles[ki] = p
                    # softmax denominators: row DH of pv holds sum_k exp.
                    # reciprocal is slow per-lane, so reshape the 1024-vector
                    # across 64 partitions, recip there, and reshape back.
                    s64 = divtmp.tile([DH + 1, QHW], f32, tag="s64")
                    nc.vector.tensor_copy(s64[DH:DH + 1, :], pv[DH:DH + 1, :])
                    rs = divtmp.tile([DH, QHW // DH], f32, tag="rs")
                    nc.gpsimd.dma_start(rs[:], s64[DH:DH + 1, :])
                    rr = divtmp.tile([DH, QHW // DH], f32, tag="rr")
                    nc.vector.reciprocal(rr[:], rs[:])
                    r0 = divtmp.tile([1, QHW], f32, tag="r0")
                    nc.gpsimd.dma_start(r0[:], rr[:])
                    recb = divtmp.tile([DH, QHW], f32, tag="recb")
                    nc.gpsimd.partition_broadcast(recb[:], r0[:], channels=DH)
                    at = divtmp.tile([DH, QHW], f32r, tag="at")
                    nc.vector.tensor_tensor(at[:], pv[0:DH, :], recb[:], ALU.mult)
                    nc.gpsimd.dma_start(
                        attnc[h // 2][rb:rb + DH, qh * QHW:(qh + 1) * QHW], at[:])

            # output projection: out[t, e] partial = attnc^T @ wo
            attn_ps.close()
            with tc.tile_pool(name="wops", bufs=4, space="PSUM") as wops:
                for tt in range(TT):
                    for ec in range(2):
                        po = wops.tile([P, 512], f32, tag="po")
                        for dci in range(4):
                            nc.tensor.matmul(
                                po[:], attnc[dci][:, tt * P:(tt + 1) * P],
                                wo_sb[:, dci, ec * 512:(ec + 1) * 512],
                                start=(dci == 0), stop=(dci == 3))
                        ob = opool.tile([P, 512], f32, tag="ob")
                        nc.scalar.copy(ob[:], po[:])
                        nc.sync.dma_start(
                            out_d[tt * P:(tt + 1) * P, ec * 512:(ec + 1) * 512], ob[:])

    nc.compile()
    return nc


def _host_prep(hidden_states, sin, cos, Wqkv, Wo):
    hidden = np.asarray(hidden_states, dtype=np.float32)
    sin = np.asarray(sin, dtype=np.float32)
    cos = np.asarray(cos, dtype=np.float32)
    Wqkv = np.asarray(Wqkv, dtype=np.float32)
    Wo = np.asarray(Wo, dtype=np.float32)

    Wq, Wk, Wv = Wqkv[0:D], Wqkv[D:2 * D], Wqkv[2 * D:3 * D]
    cos32 = np.ascontiguousarray(cos[0, :, 0, :].T)  # [32, L]
    sin32 = np.ascontiguousarray(sin[0, :, 0, :].T)
    cosb = np.ascontiguousarray(np.tile(cos32, (4, 1)))  # [128, L]
    sinb = np.ascontiguousarray(np.tile(sin32, (4, 1)))

    hid_t = [np.ascontiguousarray(hidden[b].T).astype(np.float16) for b in range(B)]

    in_maps = []
    for core in range(NCORES):
        b, hg = core // 2, core % 2
        heads = range(hg * HL, (hg + 1) * HL)

        def grouped_t(W, scale=1.0):
            rows = []
            for xh in (0, 1):
                for h in heads:
                    rows.append(W[h * DH + xh * 32: h * DH + xh * 32 + 32])
            g = np.concatenate(rows, 0)  # [512, D]
            return np.ascontiguousarray(g.T * scale).astype(np.float16)  # [D, 512]

        wq_t = grouped_t(Wq, scale=1.0 / np.sqrt(DH))
        wk_t = grouped_t(Wk)
        wv_g = np.concatenate([Wv[h * DH:(h + 1) * DH] for h in heads], 0)
        wv_t = np.ascontiguousarray(wv_g.T).astype(np.float16)
        wo_t = np.ascontiguousarray(Wo.T[hg * EQK:(hg + 1) * EQK, :])

        in_maps.append({
            "hid": hid_t[b], "wq": wq_t, "wk": wk_t, "wv": wv_t,
            "wo": wo_t, "cosb": cosb, "sinb": sinb,
        })
    return in_maps


def kernel(hidden_states, mask, sin, cos, Wqkv, Wo, _trace=False, _tmpdir=None):
    from concourse.bass_utils import run_bass_kernel_spmd

    if "nc" not in _CACHE:
        _CACHE["nc"] = _build_bass()
    nc = _CACHE["nc"]

    in_maps = _host_prep(hidden_states, sin, cos, Wqkv, Wo)
    kwargs = {}
    if _trace:
        kwargs = dict(trace=True, trace_cores=list(range(NCORES)), tmpdir=_tmpdir)
    res = run_bass_kernel_spmd(nc, in_maps, core_ids=list(range(NCORES)), **kwargs)
    _CACHE["last_result"] = res

    out = np.empty((B, L, D), dtype=np.float32)
    for b in range(B):
        out[b] = res.results[2 * b]["out"] + res.results[2 * b + 1]["out"]
    return out
